# revision 1
# baseline (speedup 1.0000x reference)
"""Multi-head attention (b=2, l=2048, d_model=1024, h=16) on 8 trn2 NeuronCores.

Sharding: tensor-parallel over heads. Each core owns 2 heads: it computes the
QKV projections for its 128 channels (transposed layout), attention for its
heads, and a rank-128 partial of the output projection. The host sums the 8
partials and adds b_o (the tensor-parallel all-reduce, done at gather time).

On-device layout/algorithm per core (all matmuls in float32r, fp32 accumulate):
  warmup:  identity matmul burst to lift the PE HAM clock gate + a dummy exp
           to preload the ACT spline table while input DMAs run.
  phase A: QT/KT/VT [128ch, 4096tok] = W.T @ xT, streamed over 512-token
           chunks; V transposed back to natural [tok, ch] tiles via PE
           transpose, augmented with a ones column (for softmax sums).
  phase B: per (batch, 1024-q-chunk, k-tile): both heads' scoresT[k,q] =
           KT_h'-slice @ QT_h-slice back to back (disjoint PE row groups run
           concurrently); expT = exp(scoresT) on ACT (the phase pacer);
           PV accumulates [V_h | 1].T @ expT into psum [65, 1024] =
           unnormalized attnT plus softmax denominators Z.
  phase C: partial_out[tok, :] = sum_h (attnU_h.T @ Wo_h) * (1/Z_h per token);
           h0/h1 matmuls pair on PE row groups; normalization deferred to
           per-partition scales (ACT) + fused multiply-add (DVE) at PSUM
           evacuation. 1/sqrt(dh) is folded into Wq/bq on the host.
"""
import sys
import types

import numpy as np

D_MODEL = 1024
H = 16
DH = 64
B = 2
L = 2048
BL = B * L            # 4096 tokens
NCORES = 8
NKT = D_MODEL // 128  # 8 feature tiles
TCH = 512             # phase-A token chunk
NCH = BL // TCH       # 8 chunks
QC = 1024             # phase-B q chunk
NQC = L // QC         # 2 per batch
NKB = L // 128        # 16 k-tiles per batch
VSTRIDE = 2 * (DH + 1)  # per-k-tile Vaug columns: [V_h0 | 1 | V_h1 | 1]


def _register_ntff_hook():
    """Install the axon NTFF profiling hook module if the image lacks it.

    Harmless if never used; required for run_bass_kernel_spmd(trace=True)."""
    if "antenv.axon_hooks" in sys.modules:
        return
    try:
        import antenv
        mod = types.ModuleType("antenv.axon_hooks")
        holder = {}
        mod.set_axon_ntff_profile_hook = lambda h: holder.__setitem__("h", h)
        mod.get_axon_ntff_profile_hook = lambda: holder.get("h")
        sys.modules["antenv.axon_hooks"] = mod
        antenv.axon_hooks = mod
        from trn_agent_boot.trn_boot import _ntff_profile_via_ctypes
        mod.set_axon_ntff_profile_hook(
            _ntff_profile_via_ctypes("/opt/axon/libaxon_pjrt.so")
        )
    except Exception:
        pass


_NC_CACHE = {}


def _build():
    if "nc" in _NC_CACHE:
        return _NC_CACHE["nc"]
    import concourse.bacc as bacc
    import concourse.tile as tile
    import concourse.mybir as mybir

    F32 = mybir.dt.float32
    F32R = mybir.dt.float32r
    AF = mybir.ActivationFunctionType
    ALU = mybir.AluOpType

    nc = bacc.Bacc("TRN2", target_bir_lowering=False, debug=False)

    xT_d = nc.dram_tensor("xT", [D_MODEL, BL], F32R, kind="ExternalInput").ap()
    wq_d = nc.dram_tensor("wq", [128, NKT * 128], F32R, kind="ExternalInput").ap()
    wk_d = nc.dram_tensor("wk", [128, NKT * 128], F32R, kind="ExternalInput").ap()
    wv_d = nc.dram_tensor("wv", [128, NKT * 128], F32R, kind="ExternalInput").ap()
    bq_d = nc.dram_tensor("bq", [128, 1], F32, kind="ExternalInput").ap()
    bk_d = nc.dram_tensor("bk", [128, 1], F32, kind="ExternalInput").ap()
    bv_d = nc.dram_tensor("bv", [128, 1], F32, kind="ExternalInput").ap()
    wo_d = nc.dram_tensor("wo", [128, D_MODEL], F32R, kind="ExternalInput").ap()
    id_d = nc.dram_tensor("ident", [128, 128], F32R, kind="ExternalInput").ap()
    out_d = nc.dram_tensor("out", [BL, D_MODEL], F32, kind="ExternalOutput").ap()

    with tile.TileContext(nc) as tc:
        with (
            tc.tile_pool(name="weights", bufs=1) as wpool,
            tc.tile_pool(name="persist", bufs=1) as ppool,
        ):
            id_t = wpool.tile([128, 128], F32R, tag="ident")
            nc.gpsimd.dma_start(id_t[:], id_d)
            wq_t = wpool.tile([128, NKT * 128], F32R, tag="wq")
            wk_t = wpool.tile([128, NKT * 128], F32R, tag="wk")
            wv_t = wpool.tile([128, NKT * 128], F32R, tag="wv")
            bq_t = wpool.tile([128, 1], F32, tag="bq")
            bk_t = wpool.tile([128, 1], F32, tag="bk")
            bv_t = wpool.tile([128, 1], F32, tag="bv")
            wo_t = wpool.tile([128, D_MODEL], F32R, tag="wo")
            for t, d in ((wq_t, wq_d), (wk_t, wk_d), (wv_t, wv_d),
                         (bq_t, bq_d), (bk_t, bk_d), (bv_t, bv_d),
                         (wo_t, wo_d)):
                nc.gpsimd.dma_start(t[:], d)

            QT = ppool.tile([128, BL], F32R, tag="QT")
            KT = ppool.tile([128, BL], F32R, tag="KT")
            VT = ppool.tile([128, BL], F32R, tag="VT")
            Vaug = ppool.tile([128, (BL // 128) * VSTRIDE], F32R, tag="Vaug")
            attnU = [ppool.tile([128, L], F32R, tag=f"attnU{b}",
                                name=f"attnU{b}") for b in range(B)]
            zrow = [[ppool.tile([1, L], F32, tag=f"zrow{h}{b}",
                                name=f"zrow{h}{b}") for b in range(B)]
                    for h in range(2)]
            rz = [[ppool.tile([128, L // 128], F32, tag=f"rz{h}{b}",
                              name=f"rz{h}{b}") for b in range(B)]
                  for h in range(2)]
            scr = ppool.tile([1, 32], F32, tag="scr")

            nc.vector.memset(Vaug[:].bitcast(F32), 1.0)

            # ---- warmup: lift HAM clock gate + preload exp table ----
            with tc.tile_pool(name="psW", bufs=1, space="PSUM") as psW:
                wu = psW.tile([128, 512], F32, tag="wu")
                for i in range(40):
                    nc.tensor.matmul(wu[:, 0:128], id_t[:], id_t[:],
                                     start=(i == 0), stop=(i == 39))
                nc.scalar.activation(scr[:], wu[0:1, 0:32], AF.Exp)

            # ---- phase A: QKV projections (transposed) + V re-transpose ----
            with (
                tc.tile_pool(name="xin", bufs=2) as xpool,
                tc.tile_pool(name="psA", bufs=4, space="PSUM") as psA,
                tc.tile_pool(name="psT", bufs=2, space="PSUM") as psT,
            ):
                for c in range(NCH):
                    sl = slice(c * TCH, (c + 1) * TCH)
                    xt = xpool.tile([128, NKT, TCH], F32R, tag="xchunk")
                    for kt in range(NKT):
                        nc.sync.dma_start(
                            xt[:, kt, :], xT_d[kt * 128:(kt + 1) * 128, sl]
                        )
                    for w_t, b_t, dst in ((wq_t, bq_t, QT), (wk_t, bk_t, KT),
                                          (wv_t, bv_t, VT)):
                        ps = psA.tile([128, TCH], F32, tag="projps")
                        for kt in range(NKT):
                            nc.tensor.matmul(
                                ps[:], w_t[:, kt * 128:(kt + 1) * 128],
                                xt[:, kt, :],
                                start=(kt == 0), stop=(kt == NKT - 1),
                            )
                        nc.vector.tensor_scalar_add(dst[:, sl], ps[:], b_t[:, 0:1])
                    # natural-layout V for the k-tiles this chunk completed
                    for g in range(c * (TCH // 128), (c + 1) * (TCH // 128)):
                        tp = psT.tile([128, 128], F32R, tag="vtrans")
                        nc.tensor.transpose(
                            tp[:], VT[:, g * 128:(g + 1) * 128], id_t[:]
                        )
                        base = g * VSTRIDE
                        nc.vector.tensor_copy(
                            Vaug[:, base:base + DH], tp[:, 0:DH]
                        )
                        nc.vector.tensor_copy(
                            Vaug[:, base + DH + 1:base + 2 * DH + 1],
                            tp[:, DH:2 * DH],
                        )

            # ---- phase B: scoresT -> exp -> PV (ACT paces; PE kept dense) ----
            # One head at a time; scores double-buffered so the PE's runnable
            # window stays deep (enables LDWEIGHTS pull-ahead). A dedicated
            # filler bank takes dep-free full-array matmuls each k-tile so the
            # PE never shows the HAM clock gate an idle window.
            with (
                tc.tile_pool(name="expP", bufs=3) as epool,
                tc.tile_pool(name="att65P", bufs=4) as apool,
                tc.tile_pool(name="oout", bufs=3) as opool,
                tc.tile_pool(name="dram", bufs=1, space="DRAM") as dpool,
                tc.tile_pool(name="psS", bufs=2, space="PSUM") as psS,
                tc.tile_pool(name="psPV", bufs=1, space="PSUM") as psPV,
                tc.tile_pool(name="psO", bufs=1, space="PSUM") as psO,
            ):
                zscr = dpool.tile([2, BL], F32, tag="zscr")

                def emit_filler(pool, tag):
                    """Dep-free full-array matmul: keeps the HAM clock gate
                    open during ACT/evac-paced stretches."""
                    f = pool.tile([128, 512], F32, tag=tag, name="fillt")
                    nc.tensor.matmul(f[:, 0:384], id_t[:], QT[:, 0:384],
                                     start=True, stop=True)

                def emit_c_unit(rc, oc, tail, alt=False):
                    """One output-projection unit: [128 tok, 512] both heads,
                    normalized via deferred per-partition 1/Z scales.

                    During the overlap with attention (tail=False) ACT is busy
                    with exps, so both evacuation ops go to DVE. In the tail,
                    psum tiles alternate into the idle scores/PV slots for
                    pipeline depth, and fillers keep the PE clock gate open."""
                    rsl = slice(rc * 128, (rc + 1) * 128)
                    bi, lrc = rc // (L // 128), rc % (L // 128)
                    lrsl = slice(lrc * 128, (lrc + 1) * 128)
                    osl = slice(oc * 512, (oc + 1) * 512)
                    if alt:
                        ps0 = psS.tile([128, 512], F32, tag="sc", name="ps0a")
                        ps1 = psS.tile([128, 512], F32, tag="sc", name="ps1a")
                    else:
                        ps0 = psO.tile([128, 512], F32, tag="ps0", name="ps0")
                        ps1 = psO.tile([128, 512], F32, tag="ps1", name="ps1")
                    # adjacent pair: row groups 0-63 / 64-127 overlap on PE
                    nc.tensor.matmul(ps0[:], attnU[bi][0:64, lrsl],
                                     wo_t[0:64, osl], start=True, stop=True)
                    nc.tensor.matmul(ps1[:], attnU[bi][64:128, lrsl],
                                     wo_t[64:128, osl], start=True, stop=True)
                    if tail:
                        emit_filler(psPV, "pv")
                    tmp = opool.tile([128, 512], F32, tag="tmp", name="tmp")
                    if tail:
                        nc.scalar.activation(tmp[:], ps0[:], AF.Copy,
                                             scale=rz[0][bi][:, lrc:lrc + 1])
                    else:
                        nc.vector.tensor_scalar_mul(tmp[:], ps0[:],
                                                    rz[0][bi][:, lrc:lrc + 1])
                    ot = opool.tile([128, 512], F32, tag="ot", name="ot")
                    nc.vector.scalar_tensor_tensor(
                        ot[:], ps1[:], rz[1][bi][:, lrc:lrc + 1], tmp[:],
                        op0=ALU.mult, op1=ALU.add,
                    )
                    nc.sync.dma_start(out_d[rsl, osl], ot[:])

                # Output-projection units become PE keep-warm work inside the
                # ACT-paced attention stretches as soon as their inputs exist:
                # batch-0 units during (b1,h0) + (b1,h1,qc0); batch-1's first
                # half during (b1,h1,qc1); only the last 16 run in the tail.
                c_queue = []
                budget = 0.0
                # dense bridge over the phase-A -> B transition: never show
                # the HAM clock gate a low-activity window
                for _ in range(16):
                    emit_filler(psO, "ps0")
                for b in range(B):
                    for h in range(2):
                        hs = slice(h * 64, (h + 1) * 64)
                        for qc in range(NQC):
                            if b == 1 and h == 0 and qc == 0:
                                c_queue += [(rc, oc) for rc in range(16)
                                            for oc in range(2)]
                            if b == 1 and h == 1 and qc == 1:
                                c_queue += [(rc, oc) for rc in range(16, 24)
                                            for oc in range(2)]
                            rate = 1.0
                            q0 = b * L + qc * QC
                            qsl = slice(q0, q0 + QC)
                            pv = psPV.tile([65, QC], F32, tag="pv")
                            for kt in range(NKB):
                                ksl = slice(b * L + kt * 128,
                                            b * L + (kt + 1) * 128)
                                sc = psS.tile([128, QC], F32, tag="sc")
                                for hf in range(QC // 512):
                                    nc.tensor.matmul(
                                        sc[:, hf * 512:(hf + 1) * 512],
                                        KT[hs, ksl],
                                        QT[hs, q0 + hf * 512:q0 + hf * 512 + 512],
                                        start=True, stop=True,
                                    )
                                ex = epool.tile([128, QC], F32R, tag="ex")
                                nc.scalar.activation(ex[:], sc[:], AF.Exp)
                                g = b * NKB + kt
                                vb = g * VSTRIDE + h * (DH + 1)
                                for hf in range(QC // 512):
                                    nc.tensor.matmul(
                                        pv[:, hf * 512:(hf + 1) * 512],
                                        Vaug[:, vb:vb + DH + 1],
                                        ex[:, hf * 512:(hf + 1) * 512],
                                        start=(kt == 0), stop=(kt == NKB - 1),
                                    )
                                if c_queue:
                                    budget += rate
                                    if budget >= 1.0:
                                        budget -= 1.0
                                        emit_c_unit(*c_queue.pop(0), tail=False)
                                    else:
                                        emit_filler(psO, "ps0")
                                else:
                                    emit_filler(psO, "ps0")
                            # bridge the evacuation bubble at the chunk edge
                            for _ in range(8):
                                emit_filler(psO, "ps0")
                            # evacuate: one copy frees the accumulator; the
                            # attnU/Z split happens off the critical path
                            a65 = apool.tile([65, QC], F32R, tag="a65")
                            nc.vector.tensor_copy(a65[:], pv[0:65, :])
                            lqsl = slice(qc * QC, (qc + 1) * QC)
                            nc.vector.tensor_copy(
                                attnU[b][h * 64:(h + 1) * 64, lqsl], a65[0:64, :]
                            )
                            nc.vector.tensor_copy(zrow[h][b][:, lqsl],
                                                  a65[64:65, :])
                            # softmax denominators -> reciprocal columns via
                            # DRAM bounce; per q-chunk on the final stretch so
                            # its output projection can start early
                            zparts = ([lqsl] if (b == 1 and h == 1) or
                                      qc == NQC - 1 else [])
                            if b != 1 or h != 1:
                                zparts = ([slice(0, L)] if qc == NQC - 1 else [])
                            for zsl in zparts:
                                nc.sync.dma_start(
                                    zscr[h:h + 1, b * L + zsl.start:
                                         b * L + zsl.stop],
                                    zrow[h][b][:, zsl])
                                zc = ppool.tile(
                                    [128, (zsl.stop - zsl.start) // 128], F32,
                                    tag=f"zc{h}{b}{qc}", name=f"zc{h}{b}{qc}")
                                nc.sync.dma_start(
                                    zc[:],
                                    zscr[h, b * L + zsl.start:b * L + zsl.stop]
                                    .rearrange("(c p) -> p c", p=128),
                                )
                                nc.vector.reciprocal(
                                    rz[h][b][:, zsl.start // 128:
                                             zsl.stop // 128],
                                    zc[:],
                                )

                # leftover queued units, then the final batch-1 quarter
                c_tail = c_queue + [(rc, oc) for rc in range(24, BL // 128)
                                    for oc in range(2)]
                for i, u in enumerate(c_tail):
                    emit_c_unit(*u, tail=True, alt=(i % 2 == 1))

    nc.compile()
    _NC_CACHE["nc"] = nc
    return nc


def _shard_inputs(x, W_qkv, b_qkv, W_o):
    xT = np.ascontiguousarray(
        x.reshape(BL, D_MODEL).T, dtype=np.float32
    )
    ident = np.eye(128, dtype=np.float32)

    def lhsT_layout(w):
        # [D_MODEL, 128] -> [128, NKT*128] with [p, kt*128+ch] = w[kt*128+p, ch]
        return np.ascontiguousarray(
            w.reshape(NKT, 128, 128).transpose(1, 0, 2).reshape(128, NKT * 128),
            dtype=np.float32,
        )

    in_maps = []
    for c in range(NCORES):
        cs = slice(c * 128, (c + 1) * 128)
        wq = W_qkv[:, cs] * 0.125
        wk = W_qkv[:, D_MODEL:][:, cs]
        wv = W_qkv[:, 2 * D_MODEL:][:, cs]
        in_maps.append({
            "xT": xT,
            "wq": lhsT_layout(wq), "wk": lhsT_layout(wk), "wv": lhsT_layout(wv),
            "bq": np.ascontiguousarray(
                b_qkv[cs] * 0.125, dtype=np.float32).reshape(128, 1),
            "bk": np.ascontiguousarray(
                b_qkv[D_MODEL:][cs], dtype=np.float32).reshape(128, 1),
            "bv": np.ascontiguousarray(
                b_qkv[2 * D_MODEL:][cs], dtype=np.float32).reshape(128, 1),
            "wo": np.ascontiguousarray(W_o[cs, :], dtype=np.float32),
            "ident": ident,
        })
    return in_maps


def _run(inputs, trace=False, tmpdir=None):
    from concourse.bass_utils import run_bass_kernel_spmd

    _register_ntff_hook()
    nc = _build()
    in_maps = _shard_inputs(
        np.asarray(inputs["x"], dtype=np.float32),
        np.asarray(inputs["W_qkv"], dtype=np.float32),
        np.asarray(inputs["b_qkv"], dtype=np.float32),
        np.asarray(inputs["W_o"], dtype=np.float32),
    )
    res = run_bass_kernel_spmd(nc, in_maps, core_ids=list(range(NCORES)),
                               trace=trace, tmpdir=tmpdir)
    partial = np.zeros((BL, D_MODEL), dtype=np.float64)
    for c in range(NCORES):
        partial += res.results[c]["out"].astype(np.float64)
    out = (partial + np.asarray(inputs["b_o"], dtype=np.float64)).astype(np.float32)
    return out.reshape(B, L, D_MODEL), res


def kernel(**inputs) -> np.ndarray:
    out, _ = _run(inputs, trace=False)
    return out



# revision 5
# speedup vs baseline: 1.3265x; 1.3265x over previous
"""Multi-head attention (b=2, l=2048, d_model=1024, h=16) on 8 trn2 NeuronCores.

Sharding: tensor-parallel over heads. Each core owns 2 heads: QKV projections
for its 128 channels (transposed layout), attention for its heads, and a
rank-128 partial of the output projection. The host sums the 8 partials and
adds b_o (the tensor-parallel all-reduce, done at gather time).

v2 design (all matmuls bf16, fp32 PSUM accumulate; ACT-paced phase B):
  A: V projection for ALL tokens first (transposed VT), PE-transposed into
     per-head natural-layout Vaug tiles (ones column appended for softmax Z);
     then Q/K projections for batch 0 only. Biases enter as a rank-1
     bias⊗ones accumulating matmul so evacuations are plain copies (ACT).
  B: per (batch, 512-q-chunk) block, 16 k-tiles: both heads' scoresT back to
     back (disjoint PE row groups 0-63/64-127 -> concurrent); ONE exp per
     k-tile over both heads' scores [128,1024] (ACT is the pacer, ~1.15us);
     PV accumulates [V_h|1].T @ exp into per-head psum [65,512].
     Q/K projections for batch 1 and output-projection units stream into the
     PE/DVE shadow under the exp chain. Scores psum double-buffered with S
     emitted one k-tile ahead so ACT never idles.
  C: out[tok,:] = sum_h (attnU_h.T @ Wo_h) * (1/Z_h); per-head psum scaled at
     evacuation (per-partition 1/Z on DVE; ACT handles half in the tail).
     1/sqrt(dh) folded into Wq/bq on the host. Z transposed via tiny DRAM
     bounce per block. Output partials written bf16; host sums in fp64.
"""
import sys
import types

import numpy as np

D_MODEL = 1024
H = 16
DH = 64
B = 2
L = 2048
BL = B * L            # 4096 tokens
NCORES = 8
NKT = D_MODEL // 128  # 8 feature tiles
TCH = 512             # phase-A token chunk
NCH = BL // TCH       # 8 chunks
QC = 512              # phase-B q chunk
NQC = L // QC         # 4 per batch
NKB = L // 128        # 16 k-tiles per batch
NG = BL // 128        # 32 global k-tile groups
VS = DH + 1           # per-k-tile Vaug cols: [V_h | 1]


def _register_ntff_hook():
    """Install the axon NTFF profiling hook module if the image lacks it."""
    if "antenv.axon_hooks" in sys.modules:
        return
    try:
        import antenv
        mod = types.ModuleType("antenv.axon_hooks")
        holder = {}
        mod.set_axon_ntff_profile_hook = lambda h: holder.__setitem__("h", h)
        mod.get_axon_ntff_profile_hook = lambda: holder.get("h")
        sys.modules["antenv.axon_hooks"] = mod
        antenv.axon_hooks = mod
        from trn_agent_boot.trn_boot import _ntff_profile_via_ctypes
        mod.set_axon_ntff_profile_hook(
            _ntff_profile_via_ctypes("/opt/axon/libaxon_pjrt.so")
        )
    except Exception:
        pass


_NC_CACHE = {}


def _build():
    if "nc" in _NC_CACHE:
        return _NC_CACHE["nc"]
    import concourse.bacc as bacc
    import concourse.tile as tile
    import concourse.mybir as mybir

    F32 = mybir.dt.float32
    BF16 = mybir.dt.bfloat16
    AF = mybir.ActivationFunctionType
    ALU = mybir.AluOpType

    nc = bacc.Bacc("TRN2", target_bir_lowering=False, debug=False)

    xT_d = nc.dram_tensor("xT", [D_MODEL, BL], BF16, kind="ExternalInput").ap()
    wq_d = nc.dram_tensor("wq", [128, NKT * 128], BF16, kind="ExternalInput").ap()
    wk_d = nc.dram_tensor("wk", [128, NKT * 128], BF16, kind="ExternalInput").ap()
    wv_d = nc.dram_tensor("wv", [128, NKT * 128], BF16, kind="ExternalInput").ap()
    bq_d = nc.dram_tensor("bq", [1, 128], BF16, kind="ExternalInput").ap()
    bk_d = nc.dram_tensor("bk", [1, 128], BF16, kind="ExternalInput").ap()
    bv_d = nc.dram_tensor("bv", [1, 128], BF16, kind="ExternalInput").ap()
    wo_d = nc.dram_tensor("wo", [128, D_MODEL], BF16, kind="ExternalInput").ap()
    id_d = nc.dram_tensor("ident", [128, 128], BF16, kind="ExternalInput").ap()
    out_d = nc.dram_tensor("out", [BL, D_MODEL], BF16, kind="ExternalOutput").ap()

    with tile.TileContext(nc) as tc:
        with (
            tc.tile_pool(name="weights", bufs=1) as wpool,
            tc.tile_pool(name="persist", bufs=1) as ppool,
            tc.tile_pool(name="expP", bufs=3) as epool,
            tc.tile_pool(name="oout", bufs=3) as opool,
            tc.tile_pool(name="zcb", bufs=2) as zpool,
            tc.tile_pool(name="dram", bufs=1, space="DRAM") as dpool,
            tc.tile_pool(name="ps", bufs=1, space="PSUM") as psp,
        ):
            id_t = wpool.tile([128, 128], BF16, tag="ident")
            nc.gpsimd.dma_start(id_t[:], id_d)
            wq_t = wpool.tile([128, NKT * 128], BF16, tag="wq")
            wk_t = wpool.tile([128, NKT * 128], BF16, tag="wk")
            wv_t = wpool.tile([128, NKT * 128], BF16, tag="wv")
            bq_t = wpool.tile([1, 128], BF16, tag="bq")
            bk_t = wpool.tile([1, 128], BF16, tag="bk")
            bv_t = wpool.tile([1, 128], BF16, tag="bv")
            wo_t = wpool.tile([128, D_MODEL], BF16, tag="wo")
            for t, d in ((wq_t, wq_d), (wk_t, wk_d), (wv_t, wv_d),
                         (bq_t, bq_d), (bk_t, bk_d), (bv_t, bv_d),
                         (wo_t, wo_d)):
                nc.gpsimd.dma_start(t[:], d)

            xall = ppool.tile([128, NKT, BL], BF16, tag="xall")
            QT = ppool.tile([128, BL], BF16, tag="QT")
            KT = ppool.tile([128, BL], BF16, tag="KT")
            VT = ppool.tile([128, BL], BF16, tag="VT")
            Vaug = [ppool.tile([128, NG * VS], BF16, tag=f"vaug{h}",
                               name=f"vaug{h}") for h in range(2)]
            attnU = [ppool.tile([128, L], BF16, tag=f"attnU{b}",
                                name=f"attnU{b}") for b in range(B)]
            rz = [[ppool.tile([128, L // 128], F32, tag=f"rz{h}{b}",
                              name=f"rz{h}{b}") for b in range(B)]
                  for h in range(2)]
            ones_t = ppool.tile([1, TCH], BF16, tag="ones")
            scr = ppool.tile([1, 32], F32, tag="scr")
            zscr = dpool.tile([2, BL], F32, tag="zscr")

            for h in range(2):
                nc.vector.memset(Vaug[h][:], 1.0)
            nc.vector.memset(ones_t[:], 1.0)

            # ---- warmup: lift HAM clock gate + preload exp table ----
            wu = psp.tile([128, 512], F32, tag="po", name="wu", bufs=2)
            for i in range(24):
                nc.tensor.matmul(wu[:, 0:128], id_t[:], id_t[:],
                                 start=(i == 0), stop=(i == 23))
            nc.scalar.activation(scr[:], wu[0:1, 0:32], AF.Exp)

            # ---- x DMAs (per chunk x feature-tile so V can start early) ----
            for c in range(NCH):
                csl = slice(c * TCH, (c + 1) * TCH)
                for kt in range(NKT):
                    nc.sync.dma_start(
                        xall[:, kt, csl], xT_d[kt * 128:(kt + 1) * 128, csl]
                    )

            # ---- phase A0: V for all chunks + transposes; Q/K for batch 0 ----
            for c in range(NCH):
                csl = slice(c * TCH, (c + 1) * TCH)
                psv = psp.tile([128, TCH], F32, tag="po", name="psv", bufs=2)
                nc.tensor.matmul(psv[:], bv_t[:], ones_t[:],
                                 start=True, stop=False)
                for kt in range(NKT):
                    nc.tensor.matmul(
                        psv[:], wv_t[:, kt * 128:(kt + 1) * 128],
                        xall[:, kt, csl], start=False, stop=(kt == NKT - 1),
                    )
                nc.scalar.activation(VT[:, csl], psv[:], AF.Copy)
                for g in range(c * (TCH // 128), (c + 1) * (TCH // 128)):
                    tp = psp.tile([128, 128], BF16, tag="scb", name="tp", bufs=2)
                    nc.tensor.transpose(
                        tp[:], VT[:, g * 128:(g + 1) * 128], id_t[:]
                    )
                    for h in range(2):
                        nc.vector.tensor_copy(
                            Vaug[h][:, g * VS:g * VS + DH],
                            tp[:, h * DH:(h + 1) * DH],
                        )
            for w_t, b_t, dst in ((wq_t, bq_t, QT), (wk_t, bk_t, KT)):
                for c in range(NCH // 2):
                    csl = slice(c * TCH, (c + 1) * TCH)
                    psq = psp.tile([128, TCH], F32, tag="po", name="psq", bufs=2)
                    nc.tensor.matmul(psq[:], b_t[:], ones_t[:],
                                     start=True, stop=False)
                    for kt in range(NKT):
                        nc.tensor.matmul(
                            psq[:], w_t[:, kt * 128:(kt + 1) * 128],
                            xall[:, kt, csl], start=False,
                            stop=(kt == NKT - 1),
                        )
                    nc.scalar.activation(dst[:, csl], psq[:], AF.Copy)

            # ---- shadow work: Q/K batch-1 projections, then O-proj units ----
            qk_items = []
            for w_t, b_t, dst in ((wq_t, bq_t, QT), (wk_t, bk_t, KT)):
                for c in range(NCH // 2, NCH):
                    csl = slice(c * TCH, (c + 1) * TCH)
                    st = {}

                    def mk_mm(kt, st=st, w_t=w_t, b_t=b_t, csl=csl):
                        def f():
                            if kt == 0:
                                st["ps"] = psp.tile([128, TCH], F32,
                                                    tag="po", name="sps", bufs=2)
                                nc.tensor.matmul(st["ps"][:], b_t[:],
                                                 ones_t[:], start=True,
                                                 stop=False)
                            nc.tensor.matmul(
                                st["ps"][:],
                                w_t[:, kt * 128:(kt + 1) * 128],
                                xall[:, kt, csl], start=False,
                                stop=(kt == NKT - 1),
                            )
                        return f

                    def mk_ev(st=st, dst=dst, csl=csl):
                        def f():
                            nc.vector.tensor_copy(dst[:, csl], st["ps"][:])
                        return f

                    for kt in range(NKT):
                        qk_items.append(mk_mm(kt))
                    qk_items.append(mk_ev())

            def emit_unit(b, t, oc, tail=False):
                """One output-projection unit: 128 tokens x 512 out-cols,
                both heads on disjoint PE row groups, deferred 1/Z scales."""
                lrsl = slice(t * 128, (t + 1) * 128)
                osl = slice(oc * 512, (oc + 1) * 512)
                ps0 = psp.tile([128, 512], F32, tag="po", name="ps0", bufs=2)
                ps1 = psp.tile([128, 512], F32, tag="po", name="ps1", bufs=2)
                nc.tensor.matmul(ps0[:], attnU[b][0:64, lrsl],
                                 wo_t[0:64, osl], start=True, stop=True)
                nc.tensor.matmul(ps1[:], attnU[b][64:128, lrsl],
                                 wo_t[64:128, osl], start=True, stop=True)
                tmp = opool.tile([128, 512], F32, tag="tmp", name="tmp")
                if tail:
                    nc.scalar.activation(tmp[:], ps0[:], AF.Copy,
                                         scale=rz[0][b][:, t:t + 1])
                else:
                    nc.vector.tensor_scalar_mul(tmp[:], ps0[:],
                                                rz[0][b][:, t:t + 1])
                ot = opool.tile([128, 512], BF16, tag="ot", name="ot")
                nc.vector.scalar_tensor_tensor(
                    ot[:], ps1[:], rz[1][b][:, t:t + 1], tmp[:],
                    op0=ALU.mult, op1=ALU.add,
                )
                nc.sync.dma_start(out_d[b * L + t * 128:b * L + (t + 1) * 128,
                                        osl], ot[:])

            # ---- phase B: flat k-tile stream over all (b, qc) blocks ----
            blocks = [(b, qc) for b in range(B) for qc in range(NQC)]
            allS = [(b, qc, kt) for (b, qc) in blocks for kt in range(NKB)]
            sc_of = {}

            def emit_S(i):
                b, qc, kt = allS[i]
                sc = psp.tile([128, 1024], F32, tag="scb", name="sc", bufs=2)
                q0 = b * L + qc * QC
                ksl = slice(b * L + kt * 128, b * L + (kt + 1) * 128)
                for h in range(2):
                    hs = slice(h * 64, (h + 1) * 64)
                    nc.tensor.matmul(sc[:, h * 512:(h + 1) * 512],
                                     KT[hs, ksl], QT[hs, q0:q0 + QC],
                                     start=True, stop=True)
                sc_of[i] = sc

            emit_S(0)
            emit_S(1)
            unit_q = []
            pv = None
            for i, (b, qc, kt) in enumerate(allS):
                if kt == 0:
                    pv = psp.tile([128, 1024], F32, tag="pv", name="pv", bufs=1)
                sc = sc_of.pop(i)
                ex = epool.tile([128, 1024], BF16, tag="ex", name="ex")
                nc.scalar.activation(ex[:], sc[:], AF.Exp)
                if i + 2 < len(allS):
                    emit_S(i + 2)
                g = b * NKB + kt
                for h in range(2):
                    nc.tensor.matmul(
                        pv[0:VS, h * 512:(h + 1) * 512],
                        Vaug[h][:, g * VS:(g + 1) * VS],
                        ex[:, h * 512:(h + 1) * 512],
                        start=(kt == 0), stop=(kt == NKB - 1),
                    )
                # shadow: drain b1 Q/K projections first, then O-proj units
                if qk_items:
                    qk_items.pop(0)()
                    if qk_items:
                        qk_items.pop(0)()
                elif unit_q:
                    emit_unit(*unit_q.pop(0))
                if kt == NKB - 1:
                    # evacuate attn + Z, then 1/Z per 128-token tile
                    qsl = slice(qc * QC, (qc + 1) * QC)
                    for h in range(2):
                        nc.vector.tensor_copy(
                            attnU[b][h * 64:(h + 1) * 64, qsl],
                            pv[0:DH, h * 512:(h + 1) * 512],
                        )
                    zsl = slice(b * L + qc * QC, b * L + (qc + 1) * QC)
                    zsb = zpool.tile([1, 1024], F32, tag="zsb", name="zsb")
                    nc.vector.tensor_copy(zsb[:], pv[DH:DH + 1, 0:1024])
                    nc.sync.dma_start(zscr[0:2, zsl], zsb[:])
                    for h in range(2):
                        zc = zpool.tile([128, QC // 128], F32, tag="zc",
                                        name="zc")
                        nc.sync.dma_start(
                            zc[:],
                            zscr[h, zsl.start:zsl.stop]
                            .rearrange("(c p) -> p c", p=128),
                        )
                        nc.vector.reciprocal(
                            rz[h][b][:, qc * (QC // 128):
                                     (qc + 1) * (QC // 128)], zc[:])
                    unit_q += [(b, qc * (QC // 128) + t, oc)
                               for t in range(QC // 128) for oc in range(2)]

            # ---- tail: leftover output-projection units ----
            for u in unit_q:
                emit_unit(*u, tail=True)

    nc.compile()
    _NC_CACHE["nc"] = nc
    return nc


def _shard_inputs(x, W_qkv, b_qkv, W_o):
    import ml_dtypes
    bf16 = ml_dtypes.bfloat16
    xT = np.ascontiguousarray(
        x.reshape(BL, D_MODEL).T.astype(np.float32)).astype(bf16)
    ident = np.eye(128, dtype=np.float32).astype(bf16)

    def lhsT_layout(w):
        # [D_MODEL, 128] -> [128, NKT*128] with [p, kt*128+ch] = w[kt*128+p, ch]
        return np.ascontiguousarray(
            w.reshape(NKT, 128, 128).transpose(1, 0, 2).reshape(128, NKT * 128)
            .astype(np.float32)).astype(bf16)

    in_maps = []
    for c in range(NCORES):
        cs = slice(c * 128, (c + 1) * 128)
        wq = W_qkv[:, cs] * 0.125
        wk = W_qkv[:, D_MODEL:][:, cs]
        wv = W_qkv[:, 2 * D_MODEL:][:, cs]
        in_maps.append({
            "xT": xT,
            "wq": lhsT_layout(wq), "wk": lhsT_layout(wk), "wv": lhsT_layout(wv),
            "bq": (b_qkv[cs] * 0.125).astype(np.float32)
            .reshape(1, 128).astype(bf16),
            "bk": b_qkv[D_MODEL:][cs].astype(np.float32)
            .reshape(1, 128).astype(bf16),
            "bv": b_qkv[2 * D_MODEL:][cs].astype(np.float32)
            .reshape(1, 128).astype(bf16),
            "wo": np.ascontiguousarray(
                W_o[cs, :].astype(np.float32)).astype(bf16),
            "ident": ident,
        })
    return in_maps


def _run(inputs, trace=False, tmpdir=None):
    from concourse.bass_utils import run_bass_kernel_spmd

    _register_ntff_hook()
    nc = _build()
    in_maps = _shard_inputs(
        np.asarray(inputs["x"], dtype=np.float32),
        np.asarray(inputs["W_qkv"], dtype=np.float32),
        np.asarray(inputs["b_qkv"], dtype=np.float32),
        np.asarray(inputs["W_o"], dtype=np.float32),
    )
    res = run_bass_kernel_spmd(nc, in_maps, core_ids=list(range(NCORES)),
                               trace=trace, tmpdir=tmpdir)
    partial = np.zeros((BL, D_MODEL), dtype=np.float64)
    for c in range(NCORES):
        partial += np.asarray(res.results[c]["out"]).astype(np.float64)
    out = (partial + np.asarray(inputs["b_o"], dtype=np.float64)).astype(np.float32)
    return out.reshape(B, L, D_MODEL), res


def kernel(**inputs) -> np.ndarray:
    out, _ = _run(inputs, trace=False)
    return out


# revision 10
# speedup vs baseline: 1.4030x; 1.0577x over previous
"""Multi-head attention (b=2, l=2048, d_model=1024, h=16) on 8 trn2 NeuronCores.

Sharding: tensor-parallel over heads. Each core owns 2 heads: QKV projections
for its 128 channels (transposed layout), attention for its heads, and a
rank-128 partial of the output projection. The host sums the 8 partials and
adds b_o (the tensor-parallel all-reduce, done at gather time).

v2 design (all matmuls bf16, fp32 PSUM accumulate; ACT-paced phase B):
  A: V projection for ALL tokens first (transposed VT), PE-transposed into
     per-head natural-layout Vaug tiles (ones column appended for softmax Z);
     then Q/K projections for batch 0 only. Biases enter as a rank-1
     bias⊗ones accumulating matmul so evacuations are plain copies (ACT).
  B: per (batch, 512-q-chunk) block, 16 k-tiles: both heads' scoresT back to
     back (disjoint PE row groups 0-63/64-127 -> concurrent); ONE exp per
     k-tile over both heads' scores [128,1024] (ACT is the pacer, ~1.15us);
     PV accumulates [V_h|1].T @ exp into per-head psum [65,512].
     Q/K projections for batch 1 and output-projection units stream into the
     PE/DVE shadow under the exp chain. Scores psum double-buffered with S
     emitted one k-tile ahead so ACT never idles.
  C: out[tok,:] = sum_h (attnU_h.T @ Wo_h) * (1/Z_h); per-head psum scaled at
     evacuation (per-partition 1/Z on DVE; ACT handles half in the tail).
     1/sqrt(dh) folded into Wq/bq on the host. Z transposed via tiny DRAM
     bounce per block. Output partials written bf16; host sums in fp64.
"""
import sys
import types

import numpy as np

D_MODEL = 1024
H = 16
DH = 64
B = 2
L = 2048
BL = B * L            # 4096 tokens
NCORES = 8
NKT = D_MODEL // 128  # 8 feature tiles
TCH = 512             # phase-A token chunk
NCH = BL // TCH       # 8 chunks
QC = 512              # phase-B q chunk
NQC = L // QC         # 4 per batch
NKB = L // 128        # 16 k-tiles per batch
NG = BL // 128        # 32 global k-tile groups
VS = DH + 1           # per-k-tile Vaug cols: [V_h | 1]


def _register_ntff_hook():
    """Install the axon NTFF profiling hook module if the image lacks it."""
    if "antenv.axon_hooks" in sys.modules:
        return
    try:
        import antenv
        mod = types.ModuleType("antenv.axon_hooks")
        holder = {}
        mod.set_axon_ntff_profile_hook = lambda h: holder.__setitem__("h", h)
        mod.get_axon_ntff_profile_hook = lambda: holder.get("h")
        sys.modules["antenv.axon_hooks"] = mod
        antenv.axon_hooks = mod
        from trn_agent_boot.trn_boot import _ntff_profile_via_ctypes
        mod.set_axon_ntff_profile_hook(
            _ntff_profile_via_ctypes("/opt/axon/libaxon_pjrt.so")
        )
    except Exception:
        pass


_NC_CACHE = {}


def _build():
    if "nc" in _NC_CACHE:
        return _NC_CACHE["nc"]
    import concourse.bacc as bacc
    import concourse.tile as tile
    import concourse.mybir as mybir

    F32 = mybir.dt.float32
    BF16 = mybir.dt.bfloat16
    AF = mybir.ActivationFunctionType
    ALU = mybir.AluOpType

    nc = bacc.Bacc("TRN2", target_bir_lowering=False, debug=False)

    xT_d = nc.dram_tensor("xT", [B, D_MODEL, L], BF16, kind="ExternalInput").ap()
    wq_d = nc.dram_tensor("wq", [128, NKT * 128], BF16, kind="ExternalInput").ap()
    wk_d = nc.dram_tensor("wk", [128, NKT * 128], BF16, kind="ExternalInput").ap()
    wv_d = nc.dram_tensor("wv", [128, NKT * 128], BF16, kind="ExternalInput").ap()
    bq_d = nc.dram_tensor("bq", [1, 128], BF16, kind="ExternalInput").ap()
    bk_d = nc.dram_tensor("bk", [1, 128], BF16, kind="ExternalInput").ap()
    bv_d = nc.dram_tensor("bv", [1, 128], BF16, kind="ExternalInput").ap()
    wo_d = nc.dram_tensor("wo", [128, D_MODEL], BF16, kind="ExternalInput").ap()
    id_d = nc.dram_tensor("ident", [128, 128], BF16, kind="ExternalInput").ap()
    out_d = nc.dram_tensor("out", [BL, D_MODEL], BF16, kind="ExternalOutput").ap()

    with tile.TileContext(nc) as tc:
        with (
            tc.tile_pool(name="weights", bufs=1) as wpool,
            tc.tile_pool(name="persist", bufs=1) as ppool,
            tc.tile_pool(name="expP", bufs=4) as epool,
            tc.tile_pool(name="oout", bufs=3) as opool,
            tc.tile_pool(name="zcb", bufs=2) as zpool,
            tc.tile_pool(name="dram", bufs=1, space="DRAM") as dpool,
            tc.tile_pool(name="ps", bufs=1, space="PSUM") as psp,
        ):
            id_t = wpool.tile([128, 128], BF16, tag="ident")
            nc.gpsimd.dma_start(id_t[:], id_d)
            wq_t = wpool.tile([128, NKT * 128], BF16, tag="wq")
            wk_t = wpool.tile([128, NKT * 128], BF16, tag="wk")
            wv_t = wpool.tile([128, NKT * 128], BF16, tag="wv")
            bq_t = wpool.tile([1, 128], BF16, tag="bq")
            bk_t = wpool.tile([1, 128], BF16, tag="bk")
            bv_t = wpool.tile([1, 128], BF16, tag="bv")
            wo_t = wpool.tile([128, D_MODEL], BF16, tag="wo")
            for t, d in ((wq_t, wq_d), (wk_t, wk_d), (wv_t, wv_d),
                         (bq_t, bq_d), (bk_t, bk_d), (bv_t, bv_d),
                         (wo_t, wo_d)):
                nc.gpsimd.dma_start(t[:], d)

            xall = ppool.tile([128, NKT, BL], BF16, tag="xall")
            QT = ppool.tile([128, BL], BF16, tag="QT")
            KT = ppool.tile([128, BL], BF16, tag="KT")
            VT = ppool.tile([128, BL], BF16, tag="VT")
            Vaug = [ppool.tile([128, NG * VS], BF16, tag=f"vaug{h}",
                               name=f"vaug{h}") for h in range(2)]
            attnU = [ppool.tile([128, L], BF16, tag=f"attnU{b}",
                                name=f"attnU{b}") for b in range(B)]
            rz = [[ppool.tile([128, L // 128], F32, tag=f"rz{h}{b}",
                              name=f"rz{h}{b}") for b in range(B)]
                  for h in range(2)]
            ones_t = ppool.tile([1, TCH], BF16, tag="ones")
            scr = ppool.tile([1, 32], F32, tag="scr")
            zscr = dpool.tile([2, BL], F32, tag="zscr")

            for h in range(2):
                nc.vector.memset(Vaug[h][:], 1.0)
            nc.vector.memset(ones_t[:], 1.0)

            # ---- warmup: lift HAM clock gate + preload exp table ----
            wu = psp.tile([128, 512], F32, tag="po", name="wu", bufs=2)
            for i in range(24):
                nc.tensor.matmul(wu[:, 0:128], id_t[:], id_t[:],
                                 start=(i == 0), stop=(i == 23))
            nc.scalar.activation(scr[:], wu[0:1, 0:32], AF.Exp)

            # ---- x DMAs: batch-major contiguous rows, batch 0 first ----
            for b in range(B):
                for kt in range(NKT):
                    nc.sync.dma_start(
                        xall[:, kt, b * L:(b + 1) * L],
                        xT_d[b, kt * 128:(kt + 1) * 128, :],
                    )

            # ---- phase A0: V for all chunks + transposes; Q/K for batch 0 ----
            for c in range(NCH):
                csl = slice(c * TCH, (c + 1) * TCH)
                psv = psp.tile([128, TCH], F32, tag="po", name="psv", bufs=2)
                nc.tensor.matmul(psv[:], bv_t[:], ones_t[:],
                                 start=True, stop=False)
                for kt in range(NKT):
                    nc.tensor.matmul(
                        psv[:], wv_t[:, kt * 128:(kt + 1) * 128],
                        xall[:, kt, csl], start=False, stop=(kt == NKT - 1),
                    )
                nc.scalar.activation(VT[:, csl], psv[:], AF.Copy)
                for g in range(c * (TCH // 128), (c + 1) * (TCH // 128)):
                    tp = psp.tile([128, 128], BF16, tag="scb", name="tp", bufs=2)
                    nc.tensor.transpose(
                        tp[:], VT[:, g * 128:(g + 1) * 128], id_t[:]
                    )
                    for h in range(2):
                        nc.vector.tensor_copy(
                            Vaug[h][:, g * VS:g * VS + DH],
                            tp[:, h * DH:(h + 1) * DH],
                        )
            for w_t, b_t, dst in ((wq_t, bq_t, QT), (wk_t, bk_t, KT)):
                for c in range(NCH // 2):
                    csl = slice(c * TCH, (c + 1) * TCH)
                    psq = psp.tile([128, TCH], F32, tag="po", name="psq", bufs=2)
                    nc.tensor.matmul(psq[:], b_t[:], ones_t[:],
                                     start=True, stop=False)
                    for kt in range(NKT):
                        nc.tensor.matmul(
                            psq[:], w_t[:, kt * 128:(kt + 1) * 128],
                            xall[:, kt, csl], start=False,
                            stop=(kt == NKT - 1),
                        )
                    nc.scalar.activation(dst[:, csl], psq[:], AF.Copy)

            # ---- shadow work: Q/K batch-1 projections, then O-proj units ----
            qk_items = []
            for w_t, b_t, dst in ((wq_t, bq_t, QT), (wk_t, bk_t, KT)):
                for c in range(NCH // 2, NCH):
                    csl = slice(c * TCH, (c + 1) * TCH)
                    st = {}

                    def mk_mm(kt, st=st, w_t=w_t, b_t=b_t, csl=csl):
                        def f():
                            if kt == 0:
                                st["ps"] = psp.tile([128, TCH], F32,
                                                    tag="po", name="sps", bufs=2)
                                nc.tensor.matmul(st["ps"][:], b_t[:],
                                                 ones_t[:], start=True,
                                                 stop=False)
                            nc.tensor.matmul(
                                st["ps"][:],
                                w_t[:, kt * 128:(kt + 1) * 128],
                                xall[:, kt, csl], start=False,
                                stop=(kt == NKT - 1),
                            )
                        return f

                    def mk_ev(st=st, dst=dst, csl=csl):
                        def f():
                            nc.vector.tensor_copy(dst[:, csl], st["ps"][:])
                        return f

                    for kt in range(NKT):
                        qk_items.append(mk_mm(kt))
                    qk_items.append(mk_ev())

            def emit_unit(b, t, oc, tail=False):
                """One output-projection unit: 128 tokens x 512 out-cols,
                both heads on disjoint PE row groups, deferred 1/Z scales."""
                lrsl = slice(t * 128, (t + 1) * 128)
                osl = slice(oc * 512, (oc + 1) * 512)
                ps0 = psp.tile([128, 512], F32, tag="po", name="ps0", bufs=2)
                ps1 = psp.tile([128, 512], F32, tag="po", name="ps1", bufs=2)
                nc.tensor.matmul(ps0[:], attnU[b][0:64, lrsl],
                                 wo_t[0:64, osl], start=True, stop=True)
                nc.tensor.matmul(ps1[:], attnU[b][64:128, lrsl],
                                 wo_t[64:128, osl], start=True, stop=True)
                tmp = opool.tile([128, 512], F32, tag="tmp", name="tmp")
                if tail:
                    nc.scalar.activation(tmp[:], ps0[:], AF.Copy,
                                         scale=rz[0][b][:, t:t + 1])
                else:
                    nc.vector.tensor_scalar_mul(tmp[:], ps0[:],
                                                rz[0][b][:, t:t + 1])
                ot = opool.tile([128, 512], BF16, tag="ot", name="ot")
                nc.vector.scalar_tensor_tensor(
                    ot[:], ps1[:], rz[1][b][:, t:t + 1], tmp[:],
                    op0=ALU.mult, op1=ALU.add,
                )
                nc.sync.dma_start(out_d[b * L + t * 128:b * L + (t + 1) * 128,
                                        osl], ot[:])

            # ---- phase B: flat k-tile stream over all (b, qc) blocks ----
            blocks = [(b, qc) for b in range(B) for qc in range(NQC)]
            allS = [(b, qc, kt) for (b, qc) in blocks for kt in range(NKB)]
            sc_of = {}

            def emit_S(i):
                b, qc, kt = allS[i]
                sc = psp.tile([128, 1024], F32, tag="scb", name="sc", bufs=2)
                q0 = b * L + qc * QC
                ksl = slice(b * L + kt * 128, b * L + (kt + 1) * 128)
                for h in range(2):
                    hs = slice(h * 64, (h + 1) * 64)
                    nc.tensor.matmul(sc[:, h * 512:(h + 1) * 512],
                                     KT[hs, ksl], QT[hs, q0:q0 + QC],
                                     start=True, stop=True)
                sc_of[i] = sc

            emit_S(0)
            emit_S(1)
            unit_q = []
            pv = None
            for i, (b, qc, kt) in enumerate(allS):
                if kt == 0:
                    pv = psp.tile([128, 1024], F32, tag="pv", name="pv", bufs=1)
                sc = sc_of.pop(i)
                ex = epool.tile([128, 1024], BF16, tag="ex", name="ex")
                nc.scalar.activation(ex[:], sc[:], AF.Exp)
                if i + 2 < len(allS):
                    emit_S(i + 2)
                g = b * NKB + kt
                for h in range(2):
                    nc.tensor.matmul(
                        pv[0:VS, h * 512:(h + 1) * 512],
                        Vaug[h][:, g * VS:(g + 1) * VS],
                        ex[:, h * 512:(h + 1) * 512],
                        start=(kt == 0), stop=(kt == NKB - 1),
                    )
                # shadow: drain b1 Q/K projections first, then O-proj units.
                # Units carry ~1.4us of DVE evacuation, which also spikes at
                # block boundaries (attn/Z evac + reciprocal) — keep units
                # away from the boundary so the in-order PE queue never
                # blocks on a psum slot behind a DVE backlog.
                if qk_items:
                    qk_items.pop(0)()
                    if qk_items:
                        qk_items.pop(0)()
                elif unit_q and 1 < kt < NKB - 2:
                    emit_unit(*unit_q.pop(0))
                if kt == NKB - 1:
                    # Z out first (its DRAM round trip overlaps the attn evac)
                    zsl = slice(b * L + qc * QC, b * L + (qc + 1) * QC)
                    zsb = zpool.tile([1, 1024], F32, tag="zsb", name="zsb")
                    nc.vector.tensor_copy(zsb[:], pv[DH:DH + 1, 0:1024])
                    nc.sync.dma_start(zscr[0:2, zsl], zsb[:])
                    for h in range(2):
                        zc = zpool.tile([128, QC // 128], F32, tag="zc",
                                        name="zc")
                        nc.sync.dma_start(
                            zc[:],
                            zscr[h, zsl.start:zsl.stop]
                            .rearrange("(c p) -> p c", p=128),
                        )
                        nc.vector.reciprocal(
                            rz[h][b][:, qc * (QC // 128):
                                     (qc + 1) * (QC // 128)], zc[:])
                    qsl = slice(qc * QC, (qc + 1) * QC)
                    for h in range(2):
                        nc.vector.tensor_copy(
                            attnU[b][h * 64:(h + 1) * 64, qsl],
                            pv[0:DH, h * 512:(h + 1) * 512],
                        )
                    unit_q += [(b, qc * (QC // 128) + t, oc)
                               for t in range(QC // 128) for oc in range(2)]

            # ---- tail: leftover output-projection units ----
            for u in unit_q:
                emit_unit(*u, tail=True)

    nc.compile()
    _NC_CACHE["nc"] = nc
    return nc


def _shard_inputs(x, W_qkv, b_qkv, W_o):
    import ml_dtypes
    bf16 = ml_dtypes.bfloat16
    xT = np.ascontiguousarray(
        x.reshape(BL, D_MODEL).T.astype(np.float32)).astype(bf16)
    ident = np.eye(128, dtype=np.float32).astype(bf16)

    xT = np.ascontiguousarray(
        xT.reshape(D_MODEL, B, L).transpose(1, 0, 2))  # [B, D_MODEL, L]

    def lhsT_layout(w):
        # [D_MODEL, 128] -> [128, NKT*128] with [p, kt*128+ch] = w[kt*128+p, ch]
        return np.ascontiguousarray(
            w.reshape(NKT, 128, 128).transpose(1, 0, 2).reshape(128, NKT * 128)
            .astype(np.float32)).astype(bf16)

    in_maps = []
    for c in range(NCORES):
        cs = slice(c * 128, (c + 1) * 128)
        wq = W_qkv[:, cs] * 0.125
        wk = W_qkv[:, D_MODEL:][:, cs]
        wv = W_qkv[:, 2 * D_MODEL:][:, cs]
        in_maps.append({
            "xT": xT,
            "wq": lhsT_layout(wq), "wk": lhsT_layout(wk), "wv": lhsT_layout(wv),
            "bq": (b_qkv[cs] * 0.125).astype(np.float32)
            .reshape(1, 128).astype(bf16),
            "bk": b_qkv[D_MODEL:][cs].astype(np.float32)
            .reshape(1, 128).astype(bf16),
            "bv": b_qkv[2 * D_MODEL:][cs].astype(np.float32)
            .reshape(1, 128).astype(bf16),
            "wo": np.ascontiguousarray(
                W_o[cs, :].astype(np.float32)).astype(bf16),
            "ident": ident,
        })
    return in_maps


def _run(inputs, trace=False, tmpdir=None):
    from concourse.bass_utils import run_bass_kernel_spmd

    _register_ntff_hook()
    nc = _build()
    in_maps = _shard_inputs(
        np.asarray(inputs["x"], dtype=np.float32),
        np.asarray(inputs["W_qkv"], dtype=np.float32),
        np.asarray(inputs["b_qkv"], dtype=np.float32),
        np.asarray(inputs["W_o"], dtype=np.float32),
    )
    res = run_bass_kernel_spmd(nc, in_maps, core_ids=list(range(NCORES)),
                               trace=trace, tmpdir=tmpdir)
    partial = np.zeros((BL, D_MODEL), dtype=np.float64)
    for c in range(NCORES):
        partial += np.asarray(res.results[c]["out"]).astype(np.float64)
    out = (partial + np.asarray(inputs["b_o"], dtype=np.float64)).astype(np.float32)
    return out.reshape(B, L, D_MODEL), res


def kernel(**inputs) -> np.ndarray:
    out, _ = _run(inputs, trace=False)
    return out


# revision 14
# speedup vs baseline: 1.4040x; 1.0007x over previous
"""Multi-head attention (b=2, l=2048, d_model=1024, h=16) on 8 trn2 NeuronCores.

Sharding: tensor-parallel over heads. Each core owns 2 heads: QKV projections
for its 128 channels (transposed layout), attention for its heads, and a
rank-128 partial of the output projection. The host sums the 8 partials and
adds b_o (the tensor-parallel all-reduce, done at gather time).

v2 design (all matmuls bf16, fp32 PSUM accumulate; ACT-paced phase B):
  A: V projection for ALL tokens first (transposed VT), PE-transposed into
     per-head natural-layout Vaug tiles (ones column appended for softmax Z);
     then Q/K projections for batch 0 only. Biases enter as a rank-1
     bias⊗ones accumulating matmul so evacuations are plain copies (ACT).
  B: per (batch, 512-q-chunk) block, 16 k-tiles: both heads' scoresT back to
     back (disjoint PE row groups 0-63/64-127 -> concurrent); ONE exp per
     k-tile over both heads' scores [128,1024] (ACT is the pacer, ~1.15us);
     PV accumulates [V_h|1].T @ exp into per-head psum [65,512].
     Q/K projections for batch 1 and output-projection units stream into the
     PE/DVE shadow under the exp chain. Scores psum double-buffered with S
     emitted one k-tile ahead so ACT never idles.
  C: out[tok,:] = sum_h (attnU_h.T @ Wo_h) * (1/Z_h); per-head psum scaled at
     evacuation (per-partition 1/Z on DVE; ACT handles half in the tail).
     1/sqrt(dh) folded into Wq/bq on the host. Z transposed via tiny DRAM
     bounce per block. Output partials written bf16; host sums in fp64.
"""
import sys
import types

import numpy as np

D_MODEL = 1024
H = 16
DH = 64
B = 2
L = 2048
BL = B * L            # 4096 tokens
NCORES = 8
NKT = D_MODEL // 128  # 8 feature tiles
TCH = 512             # phase-A token chunk
NCH = BL // TCH       # 8 chunks
QC = 512              # phase-B q chunk
NQC = L // QC         # 4 per batch
NKB = L // 128        # 16 k-tiles per batch
NG = BL // 128        # 32 global k-tile groups
VS = DH + 1           # per-k-tile Vaug cols: [V_h | 1]


def _register_ntff_hook():
    """Install the axon NTFF profiling hook module if the image lacks it."""
    if "antenv.axon_hooks" in sys.modules:
        return
    try:
        import antenv
        mod = types.ModuleType("antenv.axon_hooks")
        holder = {}
        mod.set_axon_ntff_profile_hook = lambda h: holder.__setitem__("h", h)
        mod.get_axon_ntff_profile_hook = lambda: holder.get("h")
        sys.modules["antenv.axon_hooks"] = mod
        antenv.axon_hooks = mod
        from trn_agent_boot.trn_boot import _ntff_profile_via_ctypes
        mod.set_axon_ntff_profile_hook(
            _ntff_profile_via_ctypes("/opt/axon/libaxon_pjrt.so")
        )
    except Exception:
        pass


_NC_CACHE = {}


def _build():
    if "nc" in _NC_CACHE:
        return _NC_CACHE["nc"]
    import concourse.bacc as bacc
    import concourse.tile as tile
    import concourse.mybir as mybir

    F32 = mybir.dt.float32
    BF16 = mybir.dt.bfloat16
    AF = mybir.ActivationFunctionType
    ALU = mybir.AluOpType

    nc = bacc.Bacc("TRN2", target_bir_lowering=False, debug=False)

    xT_d = nc.dram_tensor("xT", [B, D_MODEL, L], BF16, kind="ExternalInput").ap()
    wq_d = nc.dram_tensor("wq", [128, NKT * 128], BF16, kind="ExternalInput").ap()
    wk_d = nc.dram_tensor("wk", [128, NKT * 128], BF16, kind="ExternalInput").ap()
    wv_d = nc.dram_tensor("wv", [128, NKT * 128], BF16, kind="ExternalInput").ap()
    bq_d = nc.dram_tensor("bq", [1, 128], BF16, kind="ExternalInput").ap()
    bk_d = nc.dram_tensor("bk", [1, 128], BF16, kind="ExternalInput").ap()
    bv_d = nc.dram_tensor("bv", [1, 128], BF16, kind="ExternalInput").ap()
    wo_d = nc.dram_tensor("wo", [128, D_MODEL], BF16, kind="ExternalInput").ap()
    id_d = nc.dram_tensor("ident", [128, 128], BF16, kind="ExternalInput").ap()
    out_d = nc.dram_tensor("out", [BL, D_MODEL], BF16, kind="ExternalOutput").ap()

    with tile.TileContext(nc) as tc:
        with (
            tc.tile_pool(name="weights", bufs=1) as wpool,
            tc.tile_pool(name="persist", bufs=1) as ppool,
            tc.tile_pool(name="expP", bufs=4) as epool,
            tc.tile_pool(name="oout", bufs=3) as opool,
            tc.tile_pool(name="zcb", bufs=2) as zpool,
            tc.tile_pool(name="dram", bufs=1, space="DRAM") as dpool,
            tc.tile_pool(name="ps", bufs=1, space="PSUM") as psp,
        ):
            id_t = wpool.tile([128, 128], BF16, tag="ident")
            nc.gpsimd.dma_start(id_t[:], id_d)
            wq_t = wpool.tile([128, NKT * 128], BF16, tag="wq")
            wk_t = wpool.tile([128, NKT * 128], BF16, tag="wk")
            wv_t = wpool.tile([128, NKT * 128], BF16, tag="wv")
            bq_t = wpool.tile([1, 128], BF16, tag="bq")
            bk_t = wpool.tile([1, 128], BF16, tag="bk")
            bv_t = wpool.tile([1, 128], BF16, tag="bv")
            wo_t = wpool.tile([128, D_MODEL], BF16, tag="wo")
            for t, d in ((wq_t, wq_d), (wk_t, wk_d), (wv_t, wv_d),
                         (bq_t, bq_d), (bk_t, bk_d), (bv_t, bv_d),
                         (wo_t, wo_d)):
                nc.gpsimd.dma_start(t[:], d)

            xall = ppool.tile([128, NKT, BL], BF16, tag="xall")
            QT = ppool.tile([128, BL], BF16, tag="QT")
            KT = ppool.tile([128, BL], BF16, tag="KT")
            VT = ppool.tile([128, BL], BF16, tag="VT")
            Vaug = [ppool.tile([128, NG * VS], BF16, tag=f"vaug{h}",
                               name=f"vaug{h}") for h in range(2)]
            attnU = [ppool.tile([128, L], BF16, tag=f"attnU{b}",
                                name=f"attnU{b}") for b in range(B)]
            rz = [[ppool.tile([128, L // 128], F32, tag=f"rz{h}{b}",
                              name=f"rz{h}{b}") for b in range(B)]
                  for h in range(2)]
            ones_t = ppool.tile([1, TCH], BF16, tag="ones")
            scr = ppool.tile([1, 32], F32, tag="scr")
            zscr = dpool.tile([2, BL], F32, tag="zscr")

            for h in range(2):
                nc.vector.memset(Vaug[h][:], 1.0)
            nc.vector.memset(ones_t[:], 1.0)

            # ---- warmup: lift HAM clock gate + preload exp table ----
            # warmup sized to cover the first x DMA wait (~8us cold)
            wu = psp.tile([128, 512], F32, tag="po", name="wu", bufs=2)
            for i in range(72):
                nc.tensor.matmul(wu[:, 0:128], id_t[:], id_t[:],
                                 start=(i == 0), stop=(i == 71))
            nc.scalar.activation(scr[:], wu[0:1, 0:32], AF.Exp)

            # ---- x DMAs: batch-major contiguous rows; batch 0 split in
            # halves so the first V chunk can start sooner ----
            for kt in range(NKT):
                for hf in range(2):
                    nc.sync.dma_start(
                        xall[:, kt, hf * 1024:(hf + 1) * 1024],
                        xT_d[0, kt * 128:(kt + 1) * 128,
                             hf * 1024:(hf + 1) * 1024],
                    )
            for kt in range(NKT):
                nc.sync.dma_start(
                    xall[:, kt, L:2 * L], xT_d[1, kt * 128:(kt + 1) * 128, :]
                )

            # ---- phase A0: V for all chunks + transposes; Q/K for batch 0 ----
            for c in range(NCH):
                csl = slice(c * TCH, (c + 1) * TCH)
                psv = psp.tile([128, TCH], F32, tag="po", name="psv", bufs=2)
                nc.tensor.matmul(psv[:], bv_t[:], ones_t[:],
                                 start=True, stop=False)
                for kt in range(NKT):
                    nc.tensor.matmul(
                        psv[:], wv_t[:, kt * 128:(kt + 1) * 128],
                        xall[:, kt, csl], start=False, stop=(kt == NKT - 1),
                    )
                nc.scalar.activation(VT[:, csl], psv[:], AF.Copy)
                for g in range(c * (TCH // 128), (c + 1) * (TCH // 128)):
                    tp = psp.tile([128, 128], BF16, tag="scb", name="tp", bufs=2)
                    nc.tensor.transpose(
                        tp[:], VT[:, g * 128:(g + 1) * 128], id_t[:]
                    )
                    for h in range(2):
                        nc.vector.tensor_copy(
                            Vaug[h][:, g * VS:g * VS + DH],
                            tp[:, h * DH:(h + 1) * DH],
                        )
            for w_t, b_t, dst in ((wq_t, bq_t, QT), (wk_t, bk_t, KT)):
                for c in range(NCH // 2):
                    csl = slice(c * TCH, (c + 1) * TCH)
                    psq = psp.tile([128, TCH], F32, tag="po", name="psq", bufs=2)
                    nc.tensor.matmul(psq[:], b_t[:], ones_t[:],
                                     start=True, stop=False)
                    for kt in range(NKT):
                        nc.tensor.matmul(
                            psq[:], w_t[:, kt * 128:(kt + 1) * 128],
                            xall[:, kt, csl], start=False,
                            stop=(kt == NKT - 1),
                        )
                    nc.scalar.activation(dst[:, csl], psq[:], AF.Copy)

            # ---- shadow work: Q/K batch-1 projections, then O-proj units ----
            qk_items = []
            for w_t, b_t, dst in ((wq_t, bq_t, QT), (wk_t, bk_t, KT)):
                for c in range(NCH // 2, NCH):
                    csl = slice(c * TCH, (c + 1) * TCH)
                    st = {}

                    def mk_mm(kt, st=st, w_t=w_t, b_t=b_t, csl=csl):
                        def f():
                            if kt == 0:
                                st["ps"] = psp.tile([128, TCH], F32,
                                                    tag="po", name="sps", bufs=2)
                                nc.tensor.matmul(st["ps"][:], b_t[:],
                                                 ones_t[:], start=True,
                                                 stop=False)
                            nc.tensor.matmul(
                                st["ps"][:],
                                w_t[:, kt * 128:(kt + 1) * 128],
                                xall[:, kt, csl], start=False,
                                stop=(kt == NKT - 1),
                            )
                        return f

                    def mk_ev(st=st, dst=dst, csl=csl):
                        def f():
                            nc.vector.tensor_copy(dst[:, csl], st["ps"][:])
                        return f

                    for kt in range(NKT):
                        qk_items.append(mk_mm(kt))
                    qk_items.append(mk_ev())

            def emit_unit(b, t, oc, tail=False):
                """One output-projection unit: 128 tokens x 512 out-cols,
                both heads on disjoint PE row groups, deferred 1/Z scales."""
                lrsl = slice(t * 128, (t + 1) * 128)
                osl = slice(oc * 512, (oc + 1) * 512)
                ps0 = psp.tile([128, 512], F32, tag="po", name="ps0", bufs=2)
                ps1 = psp.tile([128, 512], F32, tag="po", name="ps1", bufs=2)
                nc.tensor.matmul(ps0[:], attnU[b][0:64, lrsl],
                                 wo_t[0:64, osl], start=True, stop=True)
                nc.tensor.matmul(ps1[:], attnU[b][64:128, lrsl],
                                 wo_t[64:128, osl], start=True, stop=True)
                tmp = opool.tile([128, 512], F32, tag="tmp", name="tmp")
                if tail:
                    nc.scalar.activation(tmp[:], ps0[:], AF.Copy,
                                         scale=rz[0][b][:, t:t + 1])
                else:
                    nc.vector.tensor_scalar_mul(tmp[:], ps0[:],
                                                rz[0][b][:, t:t + 1])
                ot = opool.tile([128, 512], BF16, tag="ot", name="ot")
                nc.vector.scalar_tensor_tensor(
                    ot[:], ps1[:], rz[1][b][:, t:t + 1], tmp[:],
                    op0=ALU.mult, op1=ALU.add,
                )
                nc.sync.dma_start(out_d[b * L + t * 128:b * L + (t + 1) * 128,
                                        osl], ot[:])

            # ---- phase B: flat k-tile stream over all (b, qc) blocks ----
            blocks = [(b, qc) for b in range(B) for qc in range(NQC)]
            allS = [(b, qc, kt) for (b, qc) in blocks for kt in range(NKB)]
            sc_of = {}

            def emit_S(i):
                b, qc, kt = allS[i]
                sc = psp.tile([128, 1024], F32, tag="scb", name="sc", bufs=2)
                q0 = b * L + qc * QC
                ksl = slice(b * L + kt * 128, b * L + (kt + 1) * 128)
                for h in range(2):
                    hs = slice(h * 64, (h + 1) * 64)
                    nc.tensor.matmul(sc[:, h * 512:(h + 1) * 512],
                                     KT[hs, ksl], QT[hs, q0:q0 + QC],
                                     start=True, stop=True)
                sc_of[i] = sc

            emit_S(0)
            emit_S(1)
            unit_q = []
            pv = None
            for i, (b, qc, kt) in enumerate(allS):
                if kt == 0:
                    pv = psp.tile([128, 1024], F32, tag="pv", name="pv", bufs=1)
                sc = sc_of.pop(i)
                ex = epool.tile([128, 1024], BF16, tag="ex", name="ex")
                nc.scalar.activation(ex[:], sc[:], AF.Exp)
                if i + 2 < len(allS):
                    emit_S(i + 2)
                g = b * NKB + kt

                def emit_PV():
                    for h in range(2):
                        nc.tensor.matmul(
                            pv[0:VS, h * 512:(h + 1) * 512],
                            Vaug[h][:, g * VS:(g + 1) * VS],
                            ex[:, h * 512:(h + 1) * 512],
                            start=(kt == 0), stop=(kt == NKB - 1),
                        )

                # At a block start PV(0) waits on the previous block's psum
                # evacuation (DVE); emit it last there so the next scores
                # don't queue behind it on the in-order PE.
                if kt >= 2:
                    emit_PV()
                # shadow: drain b1 Q/K projections first, then O-proj units.
                # Units carry ~1.4us of DVE evacuation, which also spikes at
                # block boundaries (attn/Z evac + reciprocal) — keep units
                # away from the boundary so the in-order PE queue never
                # blocks on a psum slot behind a DVE backlog.
                if qk_items:
                    qk_items.pop(0)()
                    if qk_items:
                        qk_items.pop(0)()
                elif unit_q and 1 < kt < NKB - 2:
                    emit_unit(*unit_q.pop(0))
                if kt < 2:
                    emit_PV()
                if kt == NKB - 1:
                    # Z out first (its DRAM round trip overlaps the attn evac)
                    zsl = slice(b * L + qc * QC, b * L + (qc + 1) * QC)
                    zsb = zpool.tile([1, 1024], F32, tag="zsb", name="zsb")
                    nc.vector.tensor_copy(zsb[:], pv[DH:DH + 1, 0:1024])
                    nc.sync.dma_start(zscr[0:2, zsl], zsb[:])
                    for h in range(2):
                        zc = zpool.tile([128, QC // 128], F32, tag="zc",
                                        name="zc")
                        nc.sync.dma_start(
                            zc[:],
                            zscr[h, zsl.start:zsl.stop]
                            .rearrange("(c p) -> p c", p=128),
                        )
                        nc.vector.reciprocal(
                            rz[h][b][:, qc * (QC // 128):
                                     (qc + 1) * (QC // 128)], zc[:])
                    qsl = slice(qc * QC, (qc + 1) * QC)
                    for h in range(2):
                        nc.vector.tensor_copy(
                            attnU[b][h * 64:(h + 1) * 64, qsl],
                            pv[0:DH, h * 512:(h + 1) * 512],
                        )
                    unit_q += [(b, qc * (QC // 128) + t, oc)
                               for t in range(QC // 128) for oc in range(2)]

            # ---- tail: leftover output-projection units; dep-free fillers
            # keep the HAM clock gate open while DVE/ACT drain ----
            for u in unit_q:
                f = psp.tile([128, 512], F32, tag="scb", name="fill", bufs=2)
                nc.tensor.matmul(f[:], id_t[:], QT[:, 0:512],
                                 start=True, stop=True)
                emit_unit(*u, tail=True)

    nc.compile()
    _NC_CACHE["nc"] = nc
    return nc


def _shard_inputs(x, W_qkv, b_qkv, W_o):
    import ml_dtypes
    bf16 = ml_dtypes.bfloat16
    xT = np.ascontiguousarray(
        x.reshape(BL, D_MODEL).T.astype(np.float32)).astype(bf16)
    ident = np.eye(128, dtype=np.float32).astype(bf16)

    xT = np.ascontiguousarray(
        xT.reshape(D_MODEL, B, L).transpose(1, 0, 2))  # [B, D_MODEL, L]

    def lhsT_layout(w):
        # [D_MODEL, 128] -> [128, NKT*128] with [p, kt*128+ch] = w[kt*128+p, ch]
        return np.ascontiguousarray(
            w.reshape(NKT, 128, 128).transpose(1, 0, 2).reshape(128, NKT * 128)
            .astype(np.float32)).astype(bf16)

    in_maps = []
    for c in range(NCORES):
        cs = slice(c * 128, (c + 1) * 128)
        wq = W_qkv[:, cs] * 0.125
        wk = W_qkv[:, D_MODEL:][:, cs]
        wv = W_qkv[:, 2 * D_MODEL:][:, cs]
        in_maps.append({
            "xT": xT,
            "wq": lhsT_layout(wq), "wk": lhsT_layout(wk), "wv": lhsT_layout(wv),
            "bq": (b_qkv[cs] * 0.125).astype(np.float32)
            .reshape(1, 128).astype(bf16),
            "bk": b_qkv[D_MODEL:][cs].astype(np.float32)
            .reshape(1, 128).astype(bf16),
            "bv": b_qkv[2 * D_MODEL:][cs].astype(np.float32)
            .reshape(1, 128).astype(bf16),
            "wo": np.ascontiguousarray(
                W_o[cs, :].astype(np.float32)).astype(bf16),
            "ident": ident,
        })
    return in_maps


def _run(inputs, trace=False, tmpdir=None):
    from concourse.bass_utils import run_bass_kernel_spmd

    _register_ntff_hook()
    nc = _build()
    in_maps = _shard_inputs(
        np.asarray(inputs["x"], dtype=np.float32),
        np.asarray(inputs["W_qkv"], dtype=np.float32),
        np.asarray(inputs["b_qkv"], dtype=np.float32),
        np.asarray(inputs["W_o"], dtype=np.float32),
    )
    res = run_bass_kernel_spmd(nc, in_maps, core_ids=list(range(NCORES)),
                               trace=trace, tmpdir=tmpdir)
    partial = np.zeros((BL, D_MODEL), dtype=np.float64)
    for c in range(NCORES):
        partial += np.asarray(res.results[c]["out"]).astype(np.float64)
    out = (partial + np.asarray(inputs["b_o"], dtype=np.float64)).astype(np.float32)
    return out.reshape(B, L, D_MODEL), res


def kernel(**inputs) -> np.ndarray:
    out, _ = _run(inputs, trace=False)
    return out


# revision 18
# speedup vs baseline: 1.4607x; 1.0404x over previous
"""Multi-head attention (b=2, l=2048, d_model=1024, h=16) on 8 trn2 NeuronCores.

Sharding: tensor-parallel over heads. Each core owns 2 heads: QKV projections
for its 128 channels (transposed layout), attention for its heads, and a
rank-128 partial of the output projection. The host sums the 8 partials and
adds b_o (the tensor-parallel all-reduce, done at gather time).

v2 design (all matmuls bf16, fp32 PSUM accumulate; ACT-paced phase B):
  A: V projection for ALL tokens first (transposed VT), PE-transposed into
     per-head natural-layout Vaug tiles (ones column appended for softmax Z);
     then Q/K projections for batch 0 only. Biases enter as a rank-1
     bias⊗ones accumulating matmul so evacuations are plain copies (ACT).
  B: per (batch, 512-q-chunk) block, 16 k-tiles: both heads' scoresT back to
     back (disjoint PE row groups 0-63/64-127 -> concurrent); ONE exp per
     k-tile over both heads' scores [128,1024] (ACT is the pacer, ~1.15us);
     PV accumulates [V_h|1].T @ exp into per-head psum [65,512].
     Q/K projections for batch 1 and output-projection units stream into the
     PE/DVE shadow under the exp chain. Scores psum double-buffered with S
     emitted one k-tile ahead so ACT never idles.
  C: out[tok,:] = sum_h (attnU_h.T @ Wo_h) * (1/Z_h); per-head psum scaled at
     evacuation (per-partition 1/Z on DVE; ACT handles half in the tail).
     1/sqrt(dh) folded into Wq/bq on the host. Z transposed via tiny DRAM
     bounce per block. Output partials written bf16; host sums in fp64.
"""
import sys
import types

import numpy as np

D_MODEL = 1024
H = 16
DH = 64
B = 2
L = 2048
BL = B * L            # 4096 tokens
NCORES = 8
NKT = D_MODEL // 128  # 8 feature tiles
TCH = 512             # phase-A token chunk
NCH = BL // TCH       # 8 chunks
QC = 512              # phase-B q chunk
NQC = L // QC         # 4 per batch
NKB = L // 128        # 16 k-tiles per batch
NG = BL // 128        # 32 global k-tile groups
VS = DH + 1           # per-k-tile Vaug cols: [V_h | 1]


def _register_ntff_hook():
    """Install the axon NTFF profiling hook module if the image lacks it."""
    if "antenv.axon_hooks" in sys.modules:
        return
    try:
        import antenv
        mod = types.ModuleType("antenv.axon_hooks")
        holder = {}
        mod.set_axon_ntff_profile_hook = lambda h: holder.__setitem__("h", h)
        mod.get_axon_ntff_profile_hook = lambda: holder.get("h")
        sys.modules["antenv.axon_hooks"] = mod
        antenv.axon_hooks = mod
        from trn_agent_boot.trn_boot import _ntff_profile_via_ctypes
        mod.set_axon_ntff_profile_hook(
            _ntff_profile_via_ctypes("/opt/axon/libaxon_pjrt.so")
        )
    except Exception:
        pass


_NC_CACHE = {}


def _build():
    if "nc" in _NC_CACHE:
        return _NC_CACHE["nc"]
    import concourse.bacc as bacc
    import concourse.tile as tile
    import concourse.mybir as mybir

    F32 = mybir.dt.float32
    BF16 = mybir.dt.bfloat16
    AF = mybir.ActivationFunctionType
    ALU = mybir.AluOpType

    nc = bacc.Bacc("TRN2", target_bir_lowering=False, debug=False)

    xT_d = nc.dram_tensor("xT", [B, D_MODEL, L], BF16, kind="ExternalInput").ap()
    wq_d = nc.dram_tensor("wq", [128, NKT * 128], BF16, kind="ExternalInput").ap()
    wk_d = nc.dram_tensor("wk", [128, NKT * 128], BF16, kind="ExternalInput").ap()
    wv_d = nc.dram_tensor("wv", [128, NKT * 128], BF16, kind="ExternalInput").ap()
    bq_d = nc.dram_tensor("bq", [128, 1], F32, kind="ExternalInput").ap()
    bk_d = nc.dram_tensor("bk", [128, 1], F32, kind="ExternalInput").ap()
    bv_d = nc.dram_tensor("bv", [128, 1], F32, kind="ExternalInput").ap()
    wo_d = nc.dram_tensor("wo", [128, D_MODEL], BF16, kind="ExternalInput").ap()
    id_d = nc.dram_tensor("ident", [128, 128], BF16, kind="ExternalInput").ap()
    out_d = nc.dram_tensor("out", [BL, D_MODEL], BF16, kind="ExternalOutput").ap()

    with tile.TileContext(nc) as tc:
        with (
            tc.tile_pool(name="weights", bufs=1) as wpool,
            tc.tile_pool(name="persist", bufs=1) as ppool,
            tc.tile_pool(name="expP", bufs=4) as epool,
            tc.tile_pool(name="oout", bufs=3) as opool,
            tc.tile_pool(name="zcb", bufs=2) as zpool,
            tc.tile_pool(name="dram", bufs=1, space="DRAM") as dpool,
            tc.tile_pool(name="ps", bufs=1, space="PSUM") as psp,
        ):
            id_t = wpool.tile([128, 128], BF16, tag="ident")
            nc.gpsimd.dma_start(id_t[:], id_d)
            wq_t = wpool.tile([128, NKT * 128], BF16, tag="wq")
            wk_t = wpool.tile([128, NKT * 128], BF16, tag="wk")
            wv_t = wpool.tile([128, NKT * 128], BF16, tag="wv")
            bq_t = wpool.tile([128, 1], F32, tag="bq")
            bk_t = wpool.tile([128, 1], F32, tag="bk")
            bv_t = wpool.tile([128, 1], F32, tag="bv")
            wo_t = wpool.tile([128, D_MODEL], BF16, tag="wo")
            for t, d in ((wq_t, wq_d), (wk_t, wk_d), (wv_t, wv_d),
                         (bq_t, bq_d), (bk_t, bk_d), (bv_t, bv_d),
                         (wo_t, wo_d)):
                nc.gpsimd.dma_start(t[:], d)

            xall = ppool.tile([128, NKT, BL], BF16, tag="xall")
            QT = ppool.tile([128, BL], BF16, tag="QT")
            KT = ppool.tile([128, BL], BF16, tag="KT")
            VT = ppool.tile([128, BL], BF16, tag="VT")
            Vaug = [ppool.tile([128, NG * VS], BF16, tag=f"vaug{h}",
                               name=f"vaug{h}") for h in range(2)]
            attnU = [ppool.tile([128, L], BF16, tag=f"attnU{b}",
                                name=f"attnU{b}") for b in range(B)]
            rz = [[ppool.tile([128, L // 128], F32, tag=f"rz{h}{b}",
                              name=f"rz{h}{b}") for b in range(B)]
                  for h in range(2)]
            scr = ppool.tile([1, 32], F32, tag="scr")
            zscr = dpool.tile([2, BL], F32, tag="zscr")

            for h in range(2):
                nc.vector.memset(Vaug[h][:], 1.0)

            # ---- warmup: lift HAM clock gate + preload exp table ----
            # warmup sized to cover the first x DMA wait (~8us cold)
            wu = psp.tile([128, 512], F32, tag="po", name="wu", bufs=2)
            for i in range(72):
                nc.tensor.matmul(wu[:, 0:128], id_t[:], id_t[:],
                                 start=(i == 0), stop=(i == 71))
            nc.scalar.activation(scr[:], wu[0:1, 0:32], AF.Exp)

            # ---- x DMAs: batch-major contiguous rows; batch 0 split in
            # halves so the first V chunk can start sooner ----
            for hf in range(2):
                for kt in range(NKT):
                    nc.sync.dma_start(
                        xall[:, kt, hf * 1024:(hf + 1) * 1024],
                        xT_d[0, kt * 128:(kt + 1) * 128,
                             hf * 1024:(hf + 1) * 1024],
                    )
            for kt in range(NKT):
                nc.sync.dma_start(
                    xall[:, kt, L:2 * L], xT_d[1, kt * 128:(kt + 1) * 128, :]
                )

            # ---- phase A0: V for all chunks + transposes; Q/K for batch 0 ----
            for c in range(NCH):
                csl = slice(c * TCH, (c + 1) * TCH)
                psv = psp.tile([128, TCH], F32, tag="po", name="psv", bufs=2)
                for kt in range(NKT):
                    nc.tensor.matmul(
                        psv[:], wv_t[:, kt * 128:(kt + 1) * 128],
                        xall[:, kt, csl], start=(kt == 0),
                        stop=(kt == NKT - 1),
                    )
                nc.vector.tensor_scalar_add(VT[:, csl], psv[:], bv_t[:, 0:1])
                for g in range(c * (TCH // 128), (c + 1) * (TCH // 128)):
                    tp = psp.tile([128, 128], BF16, tag="scb", name="tp", bufs=2)
                    nc.tensor.transpose(
                        tp[:], VT[:, g * 128:(g + 1) * 128], id_t[:]
                    )
                    for h in range(2):
                        nc.vector.tensor_copy(
                            Vaug[h][:, g * VS:g * VS + DH],
                            tp[:, h * DH:(h + 1) * DH],
                        )
            for w_t, b_t, dst in ((wq_t, bq_t, QT), (wk_t, bk_t, KT)):
                for c in range(NCH // 2):
                    csl = slice(c * TCH, (c + 1) * TCH)
                    psq = psp.tile([128, TCH], F32, tag="po", name="psq", bufs=2)
                    for kt in range(NKT):
                        nc.tensor.matmul(
                            psq[:], w_t[:, kt * 128:(kt + 1) * 128],
                            xall[:, kt, csl], start=(kt == 0),
                            stop=(kt == NKT - 1),
                        )
                    nc.vector.tensor_scalar_add(dst[:, csl], psq[:],
                                                b_t[:, 0:1])

            # ---- shadow work: Q/K batch-1 projections, then O-proj units ----
            qk_items = []
            for w_t, b_t, dst in ((wq_t, bq_t, QT), (wk_t, bk_t, KT)):
                for c in range(NCH // 2, NCH):
                    csl = slice(c * TCH, (c + 1) * TCH)
                    st = {}

                    def mk_mm(kt, st=st, w_t=w_t, csl=csl):
                        def f():
                            if kt == 0:
                                st["ps"] = psp.tile([128, TCH], F32,
                                                    tag="po", name="sps", bufs=2)
                            nc.tensor.matmul(
                                st["ps"][:],
                                w_t[:, kt * 128:(kt + 1) * 128],
                                xall[:, kt, csl], start=(kt == 0),
                                stop=(kt == NKT - 1),
                            )
                        return f

                    def mk_ev(st=st, dst=dst, b_t=b_t, csl=csl):
                        def f():
                            nc.vector.tensor_scalar_add(dst[:, csl],
                                                        st["ps"][:],
                                                        b_t[:, 0:1])
                        return f

                    for kt in range(NKT):
                        qk_items.append(mk_mm(kt))
                    qk_items.append(mk_ev())

            def emit_unit(b, t, oc, tail=False, ptag="po"):
                """One output-projection unit: 128 tokens x 512 out-cols,
                both heads on disjoint PE row groups, deferred 1/Z scales."""
                lrsl = slice(t * 128, (t + 1) * 128)
                osl = slice(oc * 512, (oc + 1) * 512)
                ps0 = psp.tile([128, 512], F32, tag=ptag, name="ps0", bufs=2)
                ps1 = psp.tile([128, 512], F32, tag=ptag, name="ps1", bufs=2)
                nc.tensor.matmul(ps0[:], attnU[b][0:64, lrsl],
                                 wo_t[0:64, osl], start=True, stop=True)
                nc.tensor.matmul(ps1[:], attnU[b][64:128, lrsl],
                                 wo_t[64:128, osl], start=True, stop=True)
                tmp = opool.tile([128, 512], F32, tag="tmp", name="tmp")
                if tail:
                    nc.scalar.activation(tmp[:], ps0[:], AF.Copy,
                                         scale=rz[0][b][:, t:t + 1])
                else:
                    nc.vector.tensor_scalar_mul(tmp[:], ps0[:],
                                                rz[0][b][:, t:t + 1])
                ot = opool.tile([128, 512], BF16, tag="ot", name="ot")
                nc.vector.scalar_tensor_tensor(
                    ot[:], ps1[:], rz[1][b][:, t:t + 1], tmp[:],
                    op0=ALU.mult, op1=ALU.add,
                )
                nc.sync.dma_start(out_d[b * L + t * 128:b * L + (t + 1) * 128,
                                        osl], ot[:])

            # ---- phase B: flat k-tile stream over all (b, qc) blocks ----
            blocks = [(b, qc) for b in range(B) for qc in range(NQC)]
            allS = [(b, qc, kt) for (b, qc) in blocks for kt in range(NKB)]
            sc_of = {}

            def emit_S(i):
                b, qc, kt = allS[i]
                sc = psp.tile([128, 1024], F32, tag="scb", name="sc", bufs=2)
                q0 = b * L + qc * QC
                ksl = slice(b * L + kt * 128, b * L + (kt + 1) * 128)
                for h in range(2):
                    hs = slice(h * 64, (h + 1) * 64)
                    nc.tensor.matmul(sc[:, h * 512:(h + 1) * 512],
                                     KT[hs, ksl], QT[hs, q0:q0 + QC],
                                     start=True, stop=True)
                sc_of[i] = sc

            emit_S(0)
            emit_S(1)
            unit_q = []
            pv_pending = []
            pv = None
            for i, (b, qc, kt) in enumerate(allS):
                if kt == 0:
                    pv = psp.tile([128, 1024], F32, tag="pv", name="pv", bufs=1)
                sc = sc_of.pop(i)
                ex = epool.tile([128, 1024], BF16, tag="ex", name="ex")
                nc.scalar.activation(ex[:], sc[:], AF.Exp)
                if i + 2 < len(allS):
                    emit_S(i + 2)
                g = b * NKB + kt

                def emit_PV(pv=pv, g=g, ex=ex, kt=kt):
                    for h in range(2):
                        nc.tensor.matmul(
                            pv[0:VS, h * 512:(h + 1) * 512],
                            Vaug[h][:, g * VS:(g + 1) * VS],
                            ex[:, h * 512:(h + 1) * 512],
                            start=(kt == 0), stop=(kt == NKB - 1),
                        )

                # At a block start PV(0)/PV(1) wait on the previous block's
                # psum evacuation (DVE); defer them past the next scores so
                # the exp chain never queues behind them on the in-order PE.
                if kt >= 2:
                    while pv_pending:
                        pv_pending.pop(0)()
                    emit_PV()
                # shadow: drain b1 Q/K projections first, then O-proj units.
                # Units carry ~1.4us of DVE evacuation, which also spikes at
                # block boundaries (attn/Z evac + reciprocal) — keep units
                # away from the boundary so the in-order PE queue never
                # blocks on a psum slot behind a DVE backlog.
                if qk_items:
                    qk_items.pop(0)()
                    if qk_items:
                        qk_items.pop(0)()
                elif unit_q and 1 < kt < NKB - 2:
                    emit_unit(*unit_q.pop(0))
                if kt < 2:
                    pv_pending.append(emit_PV)
                if kt == NKB - 1:
                    # Z out first (its DRAM round trip overlaps the attn evac)
                    zsl = slice(b * L + qc * QC, b * L + (qc + 1) * QC)
                    zsb = zpool.tile([1, 1024], F32, tag="zsb", name="zsb")
                    nc.vector.tensor_copy(zsb[:], pv[DH:DH + 1, 0:1024])
                    nc.sync.dma_start(zscr[0:2, zsl], zsb[:])
                    for h in range(2):
                        zc = zpool.tile([128, QC // 128], F32, tag="zc",
                                        name="zc")
                        nc.sync.dma_start(
                            zc[:],
                            zscr[h, zsl.start:zsl.stop]
                            .rearrange("(c p) -> p c", p=128),
                        )
                        nc.vector.reciprocal(
                            rz[h][b][:, qc * (QC // 128):
                                     (qc + 1) * (QC // 128)], zc[:])
                    qsl = slice(qc * QC, (qc + 1) * QC)
                    for h in range(2):
                        nc.vector.tensor_copy(
                            attnU[b][h * 64:(h + 1) * 64, qsl],
                            pv[0:DH, h * 512:(h + 1) * 512],
                        )
                    unit_q += [(b, qc * (QC // 128) + t, oc)
                               for t in range(QC // 128) for oc in range(2)]

            # ---- tail: leftover output-projection units. Scores psum is
            # free now, so alternate units between the po and scb rings for
            # twice the pipeline depth.
            for i, u in enumerate(unit_q):
                emit_unit(*u, tail=True, ptag=("po" if i % 2 == 0 else "scb"))

    nc.compile()
    _NC_CACHE["nc"] = nc
    return nc


def _shard_inputs(x, W_qkv, b_qkv, W_o):
    import ml_dtypes
    bf16 = ml_dtypes.bfloat16
    xT = np.ascontiguousarray(
        x.reshape(BL, D_MODEL).T.astype(np.float32)).astype(bf16)
    ident = np.eye(128, dtype=np.float32).astype(bf16)

    xT = np.ascontiguousarray(
        xT.reshape(D_MODEL, B, L).transpose(1, 0, 2))  # [B, D_MODEL, L]

    def lhsT_layout(w):
        # [D_MODEL, 128] -> [128, NKT*128] with [p, kt*128+ch] = w[kt*128+p, ch]
        return np.ascontiguousarray(
            w.reshape(NKT, 128, 128).transpose(1, 0, 2).reshape(128, NKT * 128)
            .astype(np.float32)).astype(bf16)

    in_maps = []
    for c in range(NCORES):
        cs = slice(c * 128, (c + 1) * 128)
        wq = W_qkv[:, cs] * 0.125
        wk = W_qkv[:, D_MODEL:][:, cs]
        wv = W_qkv[:, 2 * D_MODEL:][:, cs]
        in_maps.append({
            "xT": xT,
            "wq": lhsT_layout(wq), "wk": lhsT_layout(wk), "wv": lhsT_layout(wv),
            "bq": np.ascontiguousarray(
                (b_qkv[cs] * 0.125).astype(np.float32)).reshape(128, 1),
            "bk": np.ascontiguousarray(
                b_qkv[D_MODEL:][cs].astype(np.float32)).reshape(128, 1),
            "bv": np.ascontiguousarray(
                b_qkv[2 * D_MODEL:][cs].astype(np.float32)).reshape(128, 1),
            "wo": np.ascontiguousarray(
                W_o[cs, :].astype(np.float32)).astype(bf16),
            "ident": ident,
        })
    return in_maps


def _run(inputs, trace=False, tmpdir=None):
    from concourse.bass_utils import run_bass_kernel_spmd

    _register_ntff_hook()
    nc = _build()
    in_maps = _shard_inputs(
        np.asarray(inputs["x"], dtype=np.float32),
        np.asarray(inputs["W_qkv"], dtype=np.float32),
        np.asarray(inputs["b_qkv"], dtype=np.float32),
        np.asarray(inputs["W_o"], dtype=np.float32),
    )
    res = run_bass_kernel_spmd(nc, in_maps, core_ids=list(range(NCORES)),
                               trace=trace, tmpdir=tmpdir)
    partial = np.zeros((BL, D_MODEL), dtype=np.float64)
    for c in range(NCORES):
        partial += np.asarray(res.results[c]["out"]).astype(np.float64)
    out = (partial + np.asarray(inputs["b_o"], dtype=np.float64)).astype(np.float32)
    return out.reshape(B, L, D_MODEL), res


def kernel(**inputs) -> np.ndarray:
    out, _ = _run(inputs, trace=False)
    return out
